# revision 19
# baseline (speedup 1.0000x reference)
"""Discrete mixture (MoE-style routing) Bass kernel for Trainium2.

Reference computation (per batch row b):
    logits  = params[b, :K]
    gumbel  = -log(-log(uniform_noise[b]))
    sel     = argmax(logits + gumbel)                      # categorical sample
    comp    = params[b, K + sel*2D : K + (sel+1)*2D]       # gather routed expert params
    mean, log_std = comp[:D], comp[D:]
    out[b]  = mean + exp(log_std) * eps[b]

Sharding: pure data parallel over the batch axis across 8 NeuronCores
(128 rows per core, one row per SBUF partition). Each core reads only its
routing metadata (one small aux DMA), eps (1MB), and the *gathered* 2MB of
routed component params via indirect DMA — ~4MB of HBM traffic per core
instead of the 134MB full params shard.

Pipelining: the 2MB gather is split into four indirect DMAs ordered
ls0, ls1, mean0, mean1 (log_std first — it feeds exp; the last mean chunk
is small so the final add+store tail is short). Each chunk has its own
per-row offset vector (sel*2D + per-chunk base, bases packed into aux).
exp/mult run on (1024,1024) column chunks aligned to the ls gathers;
add/store run on (1536,512) chunks aligned to the mean gathers.
"""

import numpy as np

import concourse.bacc as bacc
import concourse.bass as bass
import concourse.tile as tile
from concourse import mybir
from concourse.bass_utils import run_bass_kernel_spmd

AF = mybir.ActivationFunctionType
ALU = mybir.AluOpType

B = 1024
K = 64
D = 2048
TWO_D = 2 * D
TOTAL = K + K * TWO_D  # 262208
N_CORES = 8
ROWS = B // N_CORES  # 128 rows per core == SBUF partition count

LS_SPLITS = [(0, 1024), (1024, 2048)]  # exp/mult chunks
LS_GATHERS = [(0, 2048)]  # log_std gathered whole (first in queue)
MEAN_SPLITS = [(0, 1024), (1024, 2048)]  # mean gather chunks
ADD_SPLITS = [(0, 512), (512, 1024), (1024, 1536), (1536, 2048)]  # add+store
# gather base offsets packed into aux, in issue order: ls, mean0, mean1
GATHER_BASES = [K + D + s for s, _ in LS_GATHERS] + [K + s for s, _ in MEAN_SPLITS]
N_G = len(GATHER_BASES)
AUX_W = 2 * K + N_G

_CACHE: dict = {}


def _build_program() -> bass.Bass:
    nc = bacc.Bacc("TRN2", target_bir_lowering=False, debug=False)

    params = nc.dram_tensor(
        "params", [ROWS, TOTAL], mybir.dt.float32, kind="ExternalInput"
    ).ap()
    aux = nc.dram_tensor(
        "aux", [ROWS, AUX_W], mybir.dt.uint32, kind="ExternalInput"
    ).ap()
    eps = nc.dram_tensor(
        "eps", [ROWS, D], mybir.dt.float32, kind="ExternalInput"
    ).ap()
    out = nc.dram_tensor(
        "out", [ROWS, D], mybir.dt.float32, kind="ExternalOutput"
    ).ap()

    with tile.TileContext(nc) as tc:
        with tc.tile_pool(name="p", bufs=1) as pool:
            aux_t = pool.tile([ROWS, AUX_W], mybir.dt.uint32)
            eps_t = pool.tile([ROWS, D], mybir.dt.float32)
            nc.sync.dma_start(out=aux_t[:], in_=aux[:])
            nc.sync.dma_start(out=eps_t[:], in_=eps[:])

            logits_v = aux_t[:, 0:K].bitcast(mybir.dt.float32)
            noise_v = aux_t[:, K : 2 * K].bitcast(mybir.dt.float32)
            base_v = [aux_t[:, 2 * K + i : 2 * K + i + 1] for i in range(N_G)]

            # scores = logits - log(-log(u))  (== logits + gumbel)
            t1 = pool.tile([ROWS, K], mybir.dt.float32)
            nc.scalar.activation(t1[:], noise_v, AF.Ln)
            nc.scalar.activation(t1[:], t1[:], AF.Ln, scale=-1.0)
            scores = pool.tile([ROWS, K], mybir.dt.float32)
            nc.vector.tensor_tensor(
                out=scores[:], in0=logits_v, in1=t1[:], op=ALU.subtract
            )

            # per-row argmax
            max8 = pool.tile([ROWS, 8], mybir.dt.float32)
            idx8 = pool.tile([ROWS, 8], mybir.dt.uint32)
            nc.vector.max_with_indices(max8[:], idx8[:], scores[:])

            sel4 = pool.tile([ROWS, 1], mybir.dt.uint32)
            nc.vector.tensor_scalar(
                out=sel4[:], in0=idx8[:, 0:1], scalar1=TWO_D, scalar2=None,
                op0=ALU.mult,
            )

            ls_t = pool.tile([ROWS, D], mybir.dt.float32)
            mean_t = pool.tile([ROWS, D], mybir.dt.float32)
            std = pool.tile([ROWS, D], mybir.dt.float32)
            res = pool.tile([ROWS, D], mybir.dt.float32)

            # offsets + gathers in issue order ls, mean0, mean1
            gather_dst = [ls_t[:, s:e] for s, e in LS_GATHERS] + [
                mean_t[:, s:e] for s, e in MEAN_SPLITS
            ]
            for i in range(N_G):
                offs = pool.tile([ROWS, 1], mybir.dt.uint32, name=f"offs{i}")
                nc.vector.tensor_tensor(
                    out=offs[:], in0=sel4[:], in1=base_v[i], op=ALU.add
                )
                nc.gpsimd.indirect_dma_start(
                    out=gather_dst[i],
                    out_offset=None,
                    in_=params[:, :],
                    in_offset=bass.IndirectOffsetOnAxis(ap=offs[:, 0:1], axis=1),
                )

            # exp + mult stream behind the ls gathers
            for s, e in LS_SPLITS:
                nc.scalar.activation(std[:, s:e], ls_t[:, s:e], AF.Exp)
                nc.vector.tensor_tensor(
                    out=res[:, s:e], in0=std[:, s:e], in1=eps_t[:, s:e],
                    op=ALU.mult,
                )
            # add + store in small chunks so output data streams out early
            for s, e in ADD_SPLITS:
                nc.vector.tensor_tensor(
                    out=res[:, s:e], in0=res[:, s:e], in1=mean_t[:, s:e],
                    op=ALU.add,
                )
                nc.sync.dma_start(out=out[:, s:e], in_=res[:, s:e])

    nc.finalize()
    return nc


def _get_program() -> bass.Bass:
    if "nc" not in _CACHE:
        _CACHE["nc"] = _build_program()
    return _CACHE["nc"]


def make_in_maps(params, uniform_noise, eps):
    params = np.ascontiguousarray(params, dtype=np.float32)
    uniform_noise = np.ascontiguousarray(uniform_noise, dtype=np.float32)
    eps = np.ascontiguousarray(eps, dtype=np.float32)
    row = np.arange(ROWS, dtype=np.uint64) * TOTAL
    in_maps = []
    for i in range(N_CORES):
        sl = slice(i * ROWS, (i + 1) * ROWS)
        aux = np.empty((ROWS, AUX_W), np.uint32)
        aux[:, 0:K] = np.ascontiguousarray(params[sl, :K]).view(np.uint32)
        aux[:, K : 2 * K] = uniform_noise[sl].view(np.uint32)
        for g, base in enumerate(GATHER_BASES):
            aux[:, 2 * K + g] = (row + base).astype(np.uint32)
        in_maps.append(
            {
                "params": params[sl],
                "aux": aux,
                "eps": eps[sl],
            }
        )
    return in_maps


def kernel(params, uniform_noise, eps, **run_kwargs):
    nc = _get_program()
    in_maps = make_in_maps(params, uniform_noise, eps)
    res = run_bass_kernel_spmd(nc, in_maps, list(range(N_CORES)), **run_kwargs)
    out = np.concatenate([r["out"] for r in res.results], axis=0)
    if run_kwargs:
        _CACHE["last_results"] = res
    return out
